# revision 25
# baseline (speedup 1.0000x reference)
"""YOLOv5-style ComputeLoss on 8 Trainium2 NeuronCores.

Strategy (data-parallel; dense obj plane per-image, sparse matched rows
round-robin balanced across cores):

* The loss only touches (a) the objectness channel of every cell and
  (b) all 85 channels at the <=5 matched cells around each target.
  Everything is built on the exact identity
      BCE_logits(x, y) = softplus(x) - y * x
  so each BCE sum splits into a dense softplus scan plus a sparse
  correction term over matched cells only.

* Host (numpy): YOLO build_targets-style preprocessing of the [1024, 6]
  target list, compact packing of ONLY the active (anchor, target, slot)
  rows (logits + per-slot target boxes / anchors) into small contiguous
  per-core tensors, and the final scalar reductions (including the exact
  scatter-max dedup for obj_gt and the sparse -y*x correction terms).

* Device (bass/tile, SPMD on 8 cores): contiguous DMA loads only (no
  gather), issued in parallel from otherwise-idle engines. One bf16
  input plane holds [negated box logits | obj plane | matched cls
  logits]; the Act engine runs a single table load (exp+ln share the
  natural_log_exp_and_others table) then exp over everything and
  ln(1+e) over the softplus part. Sigmoid finishes on Vector
  (1/(1+e^-x)), GIoU chain on Vector, per-level softplus sums via
  reduce_sum split across Vector (obj) and GpSimd (cls).
"""
import contextlib

import numpy as np
import ml_dtypes

import concourse.bacc as bacc
import concourse.bass as bass
import concourse.mybir as mybir
import concourse.tile as tile
from concourse import bass_utils
from concourse.hw_specs import get_activation_tables
import bass_rust as _bass_rust

NCLS = 80
ANCHOR_T = 4.0
BALANCE = (4.0, 1.0, 0.4)
HYP_BOX, HYP_CLS, HYP_OBJ = 0.05, 0.5, 1.0
_ANCHORS_PX = np.array([[10, 13, 16, 30, 33, 23],
                        [30, 61, 62, 45, 59, 119],
                        [116, 90, 156, 198, 373, 326]],
                       np.float32).reshape(3, 3, 2)
_STRIDES = np.array([8., 16., 32.], np.float32)
ANCHORS = _ANCHORS_PX / _STRIDES[:, None, None]     # [3,3,2] feature scale
LEVEL_HW = [(80, 80), (40, 40), (20, 20)]
N_IMG = 32
N_CORES = 8
IMG_PER_CORE = N_IMG // N_CORES
A = 3
EPS = 1e-7
OBJ_COLS = [600, 150, 38]     # IMG_PER_CORE*3*H*W/128 per level (lvl2 padded)
OBJ_W = sum(OBJ_COLS)         # 788
PAD_VAL = -100.0              # exp(-100) == 0 -> softplus contribution 0
F32 = mybir.dt.float32
BF16 = mybir.dt.bfloat16

# slot order: C, L, T, R, B -> (dy, dx)
SLOT_D = np.array([[0, 0], [0, -1], [-1, 0], [0, 1], [1, 0]], np.int64)

ACT_TABLE = 'natural_log_exp_and_others'


class _Bacc(bacc.Bacc):
    """Bacc that restricts activation-table selection to the exp+ln
    combo table, so the whole kernel needs exactly one table load."""

    def insert_act_table_loads(self):
        has_activation = any(
            isinstance(i, mybir.InstActivation)
            for b in self.main_func.blocks
            for i in b.instructions
        )
        if not has_activation:
            return
        tables = [(name, funcs if name == ACT_TABLE else set())
                  for name, funcs in get_activation_tables(self.m.arch).items()]
        _bass_rust.insert_act_table_loads(self, tables)


# --------------------------------------------------------------------------
# host preprocessing
# --------------------------------------------------------------------------

def _build_level(targets, lvl):
    H, W = LEVEL_HW[lvl]
    M = targets.shape[0]
    gain = np.array([1, 1, W, H, W, H], np.float32)
    t = (targets * gain).astype(np.float32)
    anc = ANCHORS[lvl]
    with np.errstate(divide='ignore', invalid='ignore'):
        r = anc[:, None, :] / t[None, :, 4:6]
        bmask = np.max(np.maximum(r, 1.0 / r), axis=2) < ANCHOR_T   # [3, M]
    bmask = bmask & np.isfinite(t[:, 4:6]).all(1)[None, :]

    img = np.clip(targets[:, 0].astype(np.int32), 0, N_IMG - 1)
    cls_id = targets[:, 1].astype(np.int32)
    cx, cy = t[:, 2], t[:, 3]
    remx, remy = cx % 1.0, cy % 1.0
    gx0 = np.floor(cx).astype(np.int64)
    gy0 = np.floor(cy).astype(np.int64)

    sl_ok = np.stack([
        np.ones(M, bool),
        (remx < 0.5) & (cx > 1.0),
        (remy < 0.5) & (cy > 1.0),
        (remx > 0.5) & (cx < W - 1.0),
        (remy > 0.5) & (cy < H - 1.0),
    ])
    cellx = np.clip(gx0[None, :] + SLOT_D[:, 1][:, None], 0, W - 1)
    celly = np.clip(gy0[None, :] + SLOT_D[:, 0][:, None], 0, H - 1)
    offs = np.array([[0., 0.], [0.5, 0.], [0., 0.5], [-0.5, 0.], [0., -0.5]],
                    np.float32)
    offx = cx[None, :] - np.floor(cx[None, :] - offs[:, 0][:, None])
    offy = cy[None, :] - np.floor(cy[None, :] - offs[:, 1][:, None])
    return dict(H=H, W=W, bmask=bmask, img=img, cls_id=cls_id,
                tw=t[:, 4], th=t[:, 5], sl_ok=sl_ok, cellx=cellx,
                celly=celly, offx=offx, offy=offy, anc=anc)


# --------------------------------------------------------------------------
# device kernel
# --------------------------------------------------------------------------

def _layout(Ts):
    """Column layout of the bf16 softplus/sigmoid input plane."""
    Tb = sum(Ts)
    B = 4 * Tb
    obj0, obj1, obj2 = B, B + 600, B + 750
    clss = B + OBJ_W
    cumT = np.concatenate([[0], np.cumsum(Ts)])
    cls_s = [clss + 80 * int(cumT[l]) for l in range(4)]
    W_SP = cls_s[3]
    # exp/DMA chunks and softplus regions (in spin column space)
    c0_end = B + 300
    chunks = [(0, c0_end), (c0_end, cls_s[1]), (cls_s[1], W_SP)]
    regions = [(obj0, obj1), (obj1, obj2), (obj2, clss),
               (cls_s[0], cls_s[1]), (cls_s[1], cls_s[2]),
               (cls_s[2], cls_s[3])]
    return Tb, B, W_SP, chunks, regions


def _build_bass(Ts):
    Tb, B, W_SP, chunks, regions = _layout(Ts)
    SPW = W_SP - B          # softplus width (obj + cls)
    OUTW = Tb + 6

    nc = _Bacc('TRN2', debug=False, num_devices=N_CORES)
    spin_d = nc.dram_tensor('spin', [128, W_SP], BF16, kind='ExternalInput')
    box_d = nc.dram_tensor('box', [128, 7 * Tb], F32, kind='ExternalInput')
    out_d = nc.dram_tensor('out', [128, OUTW], F32, kind='ExternalOutput')

    with tile.TileContext(nc) as tc:
        with contextlib.ExitStack() as ctx:
            pool = ctx.enter_context(tc.tile_pool(name='sbuf', bufs=1))
            tt = mybir.AluOpType
            af = mybir.ActivationFunctionType

            # ---- inputs: parallel-dispatch contiguous DMAs from idle engines
            spin_t = pool.tile([128, W_SP], BF16)
            box_t = pool.tile([128, 7 * Tb], F32)
            nc.sync.dma_start(spin_t[:, chunks[0][0]:chunks[0][1]],
                              spin_d.ap()[:, chunks[0][0]:chunks[0][1]])
            nc.scalar.dma_start(box_t[:], box_d.ap())
            nc.gpsimd.dma_start(spin_t[:, chunks[1][0]:chunks[1][1]],
                                spin_d.ap()[:, chunks[1][0]:chunks[1][1]])
            nc.gpsimd.dma_start(spin_t[:, chunks[2][0]:chunks[2][1]],
                                spin_d.ap()[:, chunks[2][0]:chunks[2][1]])
            out_t = pool.tile([128, OUTW], F32)

            awh4 = box_t[:, 0:2 * Tb]          # 4 * anchor wh
            tc1 = box_t[:, 2 * Tb:4 * Tb]      # target corner 1 (+0.5 shift)
            tc2 = box_t[:, 4 * Tb:6 * Tb]      # target corner 2 (+0.5 shift)
            tarea = box_t[:, 6 * Tb:7 * Tb]    # tw*th + eps

            # ---- exp over everything (one table load total)
            spe = pool.tile([128, W_SP], F32)
            for (c0, c1) in chunks:
                nc.scalar.activation(spe[:, c0:c1], spin_t[:, c0:c1], af.Exp)

            # ---- ln(1+e) over the softplus part. cls1/cls2 are the last
            #      regions the Act engine finishes, so their sums come from
            #      the fused activation accumulator (185ns read) instead of
            #      a trailing Vector reduce; obj/cls0 reduces run on Vector
            #      overlapped with the remaining Act work.
            sp = pool.tile([128, SPW], BF16)
            nc.scalar.activation(sp[:, 0:regions[3][1] - B],
                                 spe[:, B:regions[3][1]], af.Ln, bias=1.0)
            nc.scalar.activation(sp[:, regions[4][0] - B:regions[4][1] - B],
                                 spe[:, regions[4][0]:regions[4][1]],
                                 af.Ln, bias=1.0,
                                 accum_out=out_t[:, Tb + 4:Tb + 5])
            nc.scalar.activation(sp[:, regions[5][0] - B:regions[5][1] - B],
                                 spe[:, regions[5][0]:regions[5][1]],
                                 af.Ln, bias=1.0,
                                 accum_out=out_t[:, Tb + 5:Tb + 6])

            for i in (0, 1, 2, 3):
                r0, r1 = regions[i]
                nc.vector.reduce_sum(out_t[:, Tb + i:Tb + i + 1],
                                     sp[:, r0 - B:r1 - B],
                                     axis=mybir.AxisListType.X)

            # ---- sigmoid of box logits: spin holds -x, so sg = 1/(1+e^-x)
            sd = pool.tile([128, B], F32)
            nc.vector.tensor_scalar_add(sd[:], spe[:, 0:B], 1.0)
            sg = pool.tile([128, B], F32)
            nc.vector.reciprocal(sg[:], sd[:])

            # ---- GIoU on packed box columns (all positions shifted +0.5,
            #      giou is translation-invariant so the shift cancels)
            def f32t(w, tag):
                return pool.tile([128, w], F32, name=tag, tag=tag)

            sg4 = sg[:].rearrange('p (c e) -> p c e', e=4)
            pxy = f32t(2 * Tb, 'pxy')
            pxy2 = pxy[:].rearrange('p (c e) -> p c e', e=2)
            nc.vector.tensor_scalar_mul(pxy2, sg4[:, :, 0:2], 2.0)
            s2 = f32t(2 * Tb, 's2')
            s2v = s2[:].rearrange('p (c e) -> p c e', e=2)
            nc.vector.tensor_tensor(out=s2v, in0=sg4[:, :, 2:4],
                                    in1=sg4[:, :, 2:4], op=tt.mult)
            pwh = f32t(2 * Tb, 'pwh')
            nc.vector.tensor_tensor(out=pwh[:], in0=s2[:], in1=awh4,
                                    op=tt.mult)
            hwh = f32t(2 * Tb, 'hwh')
            nc.vector.tensor_scalar_mul(hwh[:], pwh[:], 0.5)
            b1 = f32t(2 * Tb, 'b1')
            nc.vector.tensor_tensor(out=b1[:], in0=pxy[:], in1=hwh[:],
                                    op=tt.subtract)
            b2 = f32t(2 * Tb, 'b2')
            nc.vector.tensor_tensor(out=b2[:], in0=pxy[:], in1=hwh[:],
                                    op=tt.add)
            i1 = f32t(2 * Tb, 'i1')
            nc.vector.tensor_tensor(out=i1[:], in0=b1[:], in1=tc1, op=tt.max)
            i2 = f32t(2 * Tb, 'i2')
            nc.vector.tensor_tensor(out=i2[:], in0=b2[:], in1=tc2, op=tt.min)
            iw = f32t(2 * Tb, 'iw')
            nc.vector.tensor_tensor(out=iw[:], in0=i2[:], in1=i1[:],
                                    op=tt.subtract)
            iwc = f32t(2 * Tb, 'iwc')
            nc.vector.tensor_scalar_max(iwc[:], iw[:], 0.0)
            c1 = f32t(2 * Tb, 'c1')
            nc.vector.tensor_tensor(out=c1[:], in0=b1[:], in1=tc1, op=tt.min)
            c2 = f32t(2 * Tb, 'c2')
            nc.vector.tensor_tensor(out=c2[:], in0=b2[:], in1=tc2, op=tt.max)
            cwh = f32t(2 * Tb, 'cwh')
            nc.vector.tensor_tensor(out=cwh[:], in0=c2[:], in1=c1[:],
                                    op=tt.subtract)

            def xy(t2):
                v = t2[:].rearrange('p (c e) -> p c e', e=2)
                return v[:, :, 0], v[:, :, 1]

            inter = f32t(Tb, 'inter')
            ix, iy = xy(iwc)
            nc.vector.tensor_tensor(out=inter[:], in0=ix, in1=iy, op=tt.mult)
            parea = f32t(Tb, 'parea')
            pwx, pwy = xy(pwh)
            nc.vector.tensor_tensor(out=parea[:], in0=pwx, in1=pwy,
                                    op=tt.mult)
            u1 = f32t(Tb, 'u1')
            nc.vector.tensor_tensor(out=u1[:], in0=parea[:], in1=tarea,
                                    op=tt.add)
            un = f32t(Tb, 'un')
            nc.vector.tensor_tensor(out=un[:], in0=u1[:], in1=inter[:],
                                    op=tt.subtract)
            ru = f32t(Tb, 'ru')
            nc.vector.reciprocal(ru[:], un[:])
            iou = f32t(Tb, 'iou')
            nc.vector.tensor_tensor(out=iou[:], in0=inter[:], in1=ru[:],
                                    op=tt.mult)
            ca0 = f32t(Tb, 'ca0')
            cwx, cwy = xy(cwh)
            nc.vector.tensor_tensor(out=ca0[:], in0=cwx, in1=cwy, op=tt.mult)
            ca = f32t(Tb, 'ca')
            nc.vector.tensor_scalar_add(ca[:], ca0[:], EPS)
            rc = f32t(Tb, 'rc')
            nc.vector.reciprocal(rc[:], ca[:])
            dif = f32t(Tb, 'dif')
            nc.vector.tensor_tensor(out=dif[:], in0=ca[:], in1=un[:],
                                    op=tt.subtract)
            dt = f32t(Tb, 'dt')
            nc.vector.tensor_tensor(out=dt[:], in0=dif[:], in1=rc[:],
                                    op=tt.mult)
            nc.vector.tensor_tensor(out=out_t[:, 0:Tb], in0=iou[:],
                                    in1=dt[:], op=tt.subtract)

            nc.scalar.dma_start(out_d.ap(), out_t[:])
    nc.compile()
    return nc


# --------------------------------------------------------------------------
# entry point
# --------------------------------------------------------------------------

def kernel(p0, p1, p2, targets):
    p0 = np.asarray(p0, np.float32)
    p1 = np.asarray(p1, np.float32)
    p2 = np.asarray(p2, np.float32)
    targets = np.asarray(targets, np.float32)
    p_list = [p0, p1, p2]
    bf16 = ml_dtypes.bfloat16

    levels = [_build_level(targets, l) for l in range(3)]

    # ---- active slot lists per level, round-robin over cores for balance
    lev = []
    for l in range(3):
        L = levels[l]
        H, W = LEVEL_HW[l]
        act = L['sl_ok'][:, None, :] & L['bmask'][None, :, :]   # [5, 3, M]
        ss, aa, mm = np.nonzero(act)
        n = len(ss)
        img = L['img'][mm]
        k = np.arange(n)
        core = k % N_CORES
        j = k // N_CORES
        celly = L['celly'][ss, mm]
        cellx = L['cellx'][ss, mm]
        p_r = p_list[l].reshape(N_IMG, A, 5 + NCLS, H, W)
        op85 = p_r[img, aa, :, celly, cellx]                    # [n, 85]
        lev.append(dict(n=n, aa=aa, img=img, core=core, j=j,
                        celly=celly, cellx=cellx, op85=op85, H=H, W=W,
                        ox=L['offx'][ss, mm], oy=L['offy'][ss, mm],
                        tw=L['tw'][mm], th=L['th'][mm],
                        anc=ANCHORS[l][aa],
                        cls_id=np.clip(L['cls_id'][mm], 0, NCLS - 1)))

    Ts = [max(1, int(-(-(-(-lev[l]['n'] // N_CORES)) // 128)))
          for l in range(3)]
    cumT = np.concatenate([[0], np.cumsum(Ts)])
    Tb = int(cumT[3])
    Tb2, B, W_SP, chunks, regions = _layout(Ts)
    assert Tb2 == Tb

    nc = _build_bass(Ts)
    OUTW = Tb + 6

    # ---- pack per-core device tensors
    spin = np.full((N_CORES, 128, W_SP), PAD_VAL, np.float32)
    boxd = np.zeros((N_CORES, 128, 7 * Tb), np.float32)
    boxd[:, :, 0:2 * Tb] = 1.0          # awh4 pad
    boxd[:, :, 4 * Tb:6 * Tb] = 1.0     # tc2 pad
    boxd[:, :, 6 * Tb:7 * Tb] = 1.0     # tarea pad

    # objectness planes (channel 4), contiguous per level
    base = B
    for l in range(3):
        H, W = LEVEL_HW[l]
        need = 128 * OBJ_COLS[l]
        for c in range(N_CORES):
            ob = np.ascontiguousarray(
                p_list[l][c * IMG_PER_CORE:(c + 1) * IMG_PER_CORE]
                .reshape(IMG_PER_CORE, A, 5 + NCLS, H, W)[:, :, 4]).reshape(-1)
            if len(ob) < need:
                ob = np.concatenate(
                    [ob, np.full(need - len(ob), PAD_VAL, np.float32)])
            spin[c, :, base:base + OBJ_COLS[l]] = ob.reshape(128, OBJ_COLS[l])
        base += OBJ_COLS[l]

    # matched-row logits + box data
    cls_s = B + OBJ_W
    for l in range(3):
        V = lev[l]
        if V['n'] == 0:
            continue
        p = V['j'] % 128
        t = V['j'] // 128
        u = cumT[l] + t
        core = V['core']
        e4 = np.arange(4)
        spin[core[:, None], p[:, None],
             u[:, None] * 4 + e4[None, :]] = -V['op85'][:, 0:4]
        cc = np.arange(NCLS)
        spin[core[:, None], p[:, None],
             cls_s + (cumT[l] + t)[:, None] * 80 + cc[None, :]] = \
            V['op85'][:, 5:]
        tw, th = V['tw'], V['th']
        ox, oy = V['ox'], V['oy']
        boxd[core, p, 2 * u] = 4.0 * V['anc'][:, 0]
        boxd[core, p, 2 * u + 1] = 4.0 * V['anc'][:, 1]
        boxd[core, p, 2 * Tb + 2 * u] = ox - tw * 0.5 + 0.5
        boxd[core, p, 2 * Tb + 2 * u + 1] = oy - th * 0.5 + 0.5
        boxd[core, p, 4 * Tb + 2 * u] = ox + tw * 0.5 + 0.5
        boxd[core, p, 4 * Tb + 2 * u + 1] = oy + th * 0.5 + 0.5
        boxd[core, p, 6 * Tb + u] = tw * th + EPS

    in_maps = [{'spin': spin[c].astype(bf16), 'box': boxd[c]}
               for c in range(N_CORES)]
    res = bass_utils.run_bass_kernel_spmd(nc, in_maps,
                                          core_ids=list(range(N_CORES)))
    global LAST_EXEC_NS, LAST_RESULT
    LAST_EXEC_NS = res.exec_time_ns
    LAST_RESULT = res
    outs = np.stack([res.results[c]['out'] for c in range(N_CORES)])

    # ---- host finalize
    total = 0.0
    for l in range(3):
        V = lev[l]
        H, W = LEVEL_HW[l]
        n = V['n']
        cnt = max(float(n), 1.0)
        obj_sum = outs[:, :, Tb + l].sum(dtype=np.float64)
        cls_sum = outs[:, :, Tb + 3 + l].sum(dtype=np.float64)
        corr = 0.0
        lbox_sum = 0.0
        xcls_sum = 0.0
        if n:
            p = V['j'] % 128
            u = cumT[l] + V['j'] // 128
            giou = outs[V['core'], p, u].astype(np.float64)
            lbox_sum = np.sum(1.0 - giou)
            xcls_sum = np.sum(V['op85'][np.arange(n), 5 + V['cls_id']]
                              .astype(np.float64))
            fk = ((V['img'].astype(np.int64) * A + V['aa']) * H
                  + V['celly']) * W + V['cellx']
            order = np.argsort(fk, kind='stable')
            fks = fk[order]
            vv = np.clip(giou[order], 0.0, None)
            xx = V['op85'][:, 4].astype(np.float64)[order]
            _, start = np.unique(fks, return_index=True)
            ymax = np.maximum.reduceat(vv, start)
            corr = np.sum(ymax * xx[start])
        count = N_IMG * A * H * W
        lb = lbox_sum / cnt
        lc = (cls_sum - xcls_sum) / (cnt * NCLS)
        lo = (obj_sum - corr) / count
        total += HYP_BOX * lb + HYP_CLS * lc + HYP_OBJ * BALANCE[l] * lo
    return np.float32(total * N_IMG)


LAST_EXEC_NS = None
LAST_RESULT = None


# revision 27
# speedup vs baseline: 1.0085x; 1.0085x over previous
"""YOLOv5-style ComputeLoss on 8 Trainium2 NeuronCores.

Strategy (data-parallel; dense obj plane per-image, sparse matched rows
round-robin balanced across cores):

* The loss only touches (a) the objectness channel of every cell and
  (b) all 85 channels at the <=5 matched cells around each target.
  Everything is built on the exact identity
      BCE_logits(x, y) = softplus(x) - y * x
  so each BCE sum splits into a dense softplus scan plus a sparse
  correction term over matched cells only.

* Host (numpy): YOLO build_targets-style preprocessing of the [1024, 6]
  target list, compact packing of ONLY the active (anchor, target, slot)
  rows (logits + per-slot target boxes / anchors) into small contiguous
  per-core tensors, and the final scalar reductions (including the exact
  scatter-max dedup for obj_gt and the sparse -y*x correction terms).

* Device (bass/tile, SPMD on 8 cores): contiguous DMA loads only (no
  gather), issued in parallel from otherwise-idle engines. One bf16
  input plane holds [negated box logits | obj plane | matched cls
  logits]; the Act engine runs a single table load (exp+ln share the
  natural_log_exp_and_others table) then exp over everything and
  ln(1+e) over the softplus part. Sigmoid finishes on Vector
  (1/(1+e^-x)), GIoU chain on Vector, per-level softplus sums via
  reduce_sum split across Vector (obj) and GpSimd (cls).
"""
import contextlib

import numpy as np
import ml_dtypes

import concourse.bacc as bacc
import concourse.bass as bass
import concourse.mybir as mybir
import concourse.tile as tile
from concourse import bass_utils
from concourse.hw_specs import get_activation_tables
import bass_rust as _bass_rust

NCLS = 80
ANCHOR_T = 4.0
BALANCE = (4.0, 1.0, 0.4)
HYP_BOX, HYP_CLS, HYP_OBJ = 0.05, 0.5, 1.0
_ANCHORS_PX = np.array([[10, 13, 16, 30, 33, 23],
                        [30, 61, 62, 45, 59, 119],
                        [116, 90, 156, 198, 373, 326]],
                       np.float32).reshape(3, 3, 2)
_STRIDES = np.array([8., 16., 32.], np.float32)
ANCHORS = _ANCHORS_PX / _STRIDES[:, None, None]     # [3,3,2] feature scale
LEVEL_HW = [(80, 80), (40, 40), (20, 20)]
N_IMG = 32
N_CORES = 8
IMG_PER_CORE = N_IMG // N_CORES
A = 3
EPS = 1e-7
OBJ_COLS = [600, 150, 38]     # IMG_PER_CORE*3*H*W/128 per level (lvl2 padded)
OBJ_W = sum(OBJ_COLS)         # 788
PAD_VAL = -100.0              # exp(-100) == 0 -> softplus contribution 0
F32 = mybir.dt.float32
BF16 = mybir.dt.bfloat16

# slot order: C, L, T, R, B -> (dy, dx)
SLOT_D = np.array([[0, 0], [0, -1], [-1, 0], [0, 1], [1, 0]], np.int64)

ACT_TABLE = 'natural_log_exp_and_others'


class _Bacc(bacc.Bacc):
    """Bacc that restricts activation-table selection to the exp+ln
    combo table, so the whole kernel needs exactly one table load."""

    def insert_act_table_loads(self):
        has_activation = any(
            isinstance(i, mybir.InstActivation)
            for b in self.main_func.blocks
            for i in b.instructions
        )
        if not has_activation:
            return
        tables = [(name, funcs if name == ACT_TABLE else set())
                  for name, funcs in get_activation_tables(self.m.arch).items()]
        _bass_rust.insert_act_table_loads(self, tables)


# --------------------------------------------------------------------------
# host preprocessing
# --------------------------------------------------------------------------

def _build_level(targets, lvl):
    H, W = LEVEL_HW[lvl]
    M = targets.shape[0]
    gain = np.array([1, 1, W, H, W, H], np.float32)
    t = (targets * gain).astype(np.float32)
    anc = ANCHORS[lvl]
    with np.errstate(divide='ignore', invalid='ignore'):
        r = anc[:, None, :] / t[None, :, 4:6]
        bmask = np.max(np.maximum(r, 1.0 / r), axis=2) < ANCHOR_T   # [3, M]
    bmask = bmask & np.isfinite(t[:, 4:6]).all(1)[None, :]

    img = np.clip(targets[:, 0].astype(np.int32), 0, N_IMG - 1)
    cls_id = targets[:, 1].astype(np.int32)
    cx, cy = t[:, 2], t[:, 3]
    remx, remy = cx % 1.0, cy % 1.0
    gx0 = np.floor(cx).astype(np.int64)
    gy0 = np.floor(cy).astype(np.int64)

    sl_ok = np.stack([
        np.ones(M, bool),
        (remx < 0.5) & (cx > 1.0),
        (remy < 0.5) & (cy > 1.0),
        (remx > 0.5) & (cx < W - 1.0),
        (remy > 0.5) & (cy < H - 1.0),
    ])
    cellx = np.clip(gx0[None, :] + SLOT_D[:, 1][:, None], 0, W - 1)
    celly = np.clip(gy0[None, :] + SLOT_D[:, 0][:, None], 0, H - 1)
    offs = np.array([[0., 0.], [0.5, 0.], [0., 0.5], [-0.5, 0.], [0., -0.5]],
                    np.float32)
    offx = cx[None, :] - np.floor(cx[None, :] - offs[:, 0][:, None])
    offy = cy[None, :] - np.floor(cy[None, :] - offs[:, 1][:, None])
    return dict(H=H, W=W, bmask=bmask, img=img, cls_id=cls_id,
                tw=t[:, 4], th=t[:, 5], sl_ok=sl_ok, cellx=cellx,
                celly=celly, offx=offx, offy=offy, anc=anc)


# --------------------------------------------------------------------------
# device kernel
# --------------------------------------------------------------------------

def _layout(Ts):
    """Column layout of the bf16 softplus/sigmoid input plane."""
    Tb = sum(Ts)
    B = 4 * Tb
    obj0, obj1, obj2 = B, B + 600, B + 750
    clss = B + OBJ_W
    cumT = np.concatenate([[0], np.cumsum(Ts)])
    cls_s = [clss + 80 * int(cumT[l]) for l in range(4)]
    W_SP = cls_s[3]
    # exp/DMA chunks and softplus regions (in spin column space)
    c0_end = B + 300
    chunks = [(0, c0_end), (c0_end, cls_s[1]), (cls_s[1], W_SP)]
    regions = [(obj0, obj1), (obj1, obj2), (obj2, clss),
               (cls_s[0], cls_s[1]), (cls_s[1], cls_s[2]),
               (cls_s[2], cls_s[3])]
    return Tb, B, W_SP, chunks, regions


def _build_bass(Ts):
    Tb, B, W_SP, chunks, regions = _layout(Ts)
    SPW = W_SP - B          # softplus width (obj + cls)
    OUTW = Tb + 6

    nc = _Bacc('TRN2', debug=False, num_devices=N_CORES)
    spin_d = nc.dram_tensor('spin', [128, W_SP], BF16, kind='ExternalInput')
    box_d = nc.dram_tensor('box', [128, 7 * Tb], F32, kind='ExternalInput')
    out_d = nc.dram_tensor('out', [128, OUTW], F32, kind='ExternalOutput')

    with tile.TileContext(nc) as tc:
        with contextlib.ExitStack() as ctx:
            pool = ctx.enter_context(tc.tile_pool(name='sbuf', bufs=1))
            tt = mybir.AluOpType
            af = mybir.ActivationFunctionType

            # ---- inputs: parallel-dispatch contiguous DMAs from idle engines
            spin_t = pool.tile([128, W_SP], BF16)
            box_t = pool.tile([128, 7 * Tb], F32)
            nc.sync.dma_start(spin_t[:, chunks[0][0]:chunks[0][1]],
                              spin_d.ap()[:, chunks[0][0]:chunks[0][1]])
            nc.scalar.dma_start(box_t[:], box_d.ap())
            nc.gpsimd.dma_start(spin_t[:, chunks[1][0]:chunks[1][1]],
                                spin_d.ap()[:, chunks[1][0]:chunks[1][1]])
            nc.gpsimd.dma_start(spin_t[:, chunks[2][0]:chunks[2][1]],
                                spin_d.ap()[:, chunks[2][0]:chunks[2][1]])
            out_t = pool.tile([128, OUTW], F32)

            awh4 = box_t[:, 0:2 * Tb]          # 4 * anchor wh
            tc1 = box_t[:, 2 * Tb:4 * Tb]      # target corner 1 (+0.5 shift)
            tc2 = box_t[:, 4 * Tb:6 * Tb]      # target corner 2 (+0.5 shift)
            tarea = box_t[:, 6 * Tb:7 * Tb]    # tw*th + eps

            # ---- exp over everything (one table load total). The ln of
            #      chunk0's obj columns is interleaved right after exp(c0):
            #      it fills the Act engine's idle window while chunk1's DMA
            #      is still in flight.
            spe = pool.tile([128, W_SP], F32)
            sp = pool.tile([128, SPW], BF16)
            nc.scalar.activation(spe[:, chunks[0][0]:chunks[0][1]],
                                 spin_t[:, chunks[0][0]:chunks[0][1]], af.Exp)
            nc.scalar.activation(sp[:, 0:chunks[0][1] - B],
                                 spe[:, B:chunks[0][1]], af.Ln, bias=1.0)
            for (c0, c1) in chunks[1:]:
                nc.scalar.activation(spe[:, c0:c1], spin_t[:, c0:c1], af.Exp)

            # ---- ln(1+e) over the softplus part. cls1/cls2 are the last
            #      regions the Act engine finishes, so their sums come from
            #      the fused activation accumulator (185ns read) instead of
            #      a trailing Vector reduce; obj/cls0 reduces run on Vector
            #      overlapped with the remaining Act work.
            nc.scalar.activation(sp[:, chunks[0][1] - B:regions[3][1] - B],
                                 spe[:, chunks[0][1]:regions[3][1]],
                                 af.Ln, bias=1.0)
            nc.scalar.activation(sp[:, regions[4][0] - B:regions[4][1] - B],
                                 spe[:, regions[4][0]:regions[4][1]],
                                 af.Ln, bias=1.0,
                                 accum_out=out_t[:, Tb + 4:Tb + 5])
            nc.scalar.activation(sp[:, regions[5][0] - B:regions[5][1] - B],
                                 spe[:, regions[5][0]:regions[5][1]],
                                 af.Ln, bias=1.0,
                                 accum_out=out_t[:, Tb + 5:Tb + 6])

            for i in (0, 1, 2, 3):
                r0, r1 = regions[i]
                nc.vector.reduce_sum(out_t[:, Tb + i:Tb + i + 1],
                                     sp[:, r0 - B:r1 - B],
                                     axis=mybir.AxisListType.X)

            # ---- sigmoid of box logits: spin holds -x, so sg = 1/(1+e^-x)
            sd = pool.tile([128, B], F32)
            nc.vector.tensor_scalar_add(sd[:], spe[:, 0:B], 1.0)
            sg = pool.tile([128, B], F32)
            nc.vector.reciprocal(sg[:], sd[:])

            # ---- GIoU on packed box columns (all positions shifted +0.5,
            #      giou is translation-invariant so the shift cancels)
            def f32t(w, tag):
                return pool.tile([128, w], F32, name=tag, tag=tag)

            sg4 = sg[:].rearrange('p (c e) -> p c e', e=4)
            pxy = f32t(2 * Tb, 'pxy')
            pxy2 = pxy[:].rearrange('p (c e) -> p c e', e=2)
            nc.vector.tensor_scalar_mul(pxy2, sg4[:, :, 0:2], 2.0)
            s2 = f32t(2 * Tb, 's2')
            s2v = s2[:].rearrange('p (c e) -> p c e', e=2)
            nc.vector.tensor_tensor(out=s2v, in0=sg4[:, :, 2:4],
                                    in1=sg4[:, :, 2:4], op=tt.mult)
            pwh = f32t(2 * Tb, 'pwh')
            nc.vector.tensor_tensor(out=pwh[:], in0=s2[:], in1=awh4,
                                    op=tt.mult)
            hwh = f32t(2 * Tb, 'hwh')
            nc.vector.tensor_scalar_mul(hwh[:], pwh[:], 0.5)
            b1 = f32t(2 * Tb, 'b1')
            nc.vector.tensor_tensor(out=b1[:], in0=pxy[:], in1=hwh[:],
                                    op=tt.subtract)
            b2 = f32t(2 * Tb, 'b2')
            nc.vector.tensor_tensor(out=b2[:], in0=pxy[:], in1=hwh[:],
                                    op=tt.add)
            i1 = f32t(2 * Tb, 'i1')
            nc.vector.tensor_tensor(out=i1[:], in0=b1[:], in1=tc1, op=tt.max)
            i2 = f32t(2 * Tb, 'i2')
            nc.vector.tensor_tensor(out=i2[:], in0=b2[:], in1=tc2, op=tt.min)
            iw = f32t(2 * Tb, 'iw')
            nc.vector.tensor_tensor(out=iw[:], in0=i2[:], in1=i1[:],
                                    op=tt.subtract)
            iwc = f32t(2 * Tb, 'iwc')
            nc.vector.tensor_scalar_max(iwc[:], iw[:], 0.0)
            c1 = f32t(2 * Tb, 'c1')
            nc.vector.tensor_tensor(out=c1[:], in0=b1[:], in1=tc1, op=tt.min)
            c2 = f32t(2 * Tb, 'c2')
            nc.vector.tensor_tensor(out=c2[:], in0=b2[:], in1=tc2, op=tt.max)
            cwh = f32t(2 * Tb, 'cwh')
            nc.vector.tensor_tensor(out=cwh[:], in0=c2[:], in1=c1[:],
                                    op=tt.subtract)

            def xy(t2):
                v = t2[:].rearrange('p (c e) -> p c e', e=2)
                return v[:, :, 0], v[:, :, 1]

            inter = f32t(Tb, 'inter')
            ix, iy = xy(iwc)
            nc.vector.tensor_tensor(out=inter[:], in0=ix, in1=iy, op=tt.mult)
            parea = f32t(Tb, 'parea')
            pwx, pwy = xy(pwh)
            nc.vector.tensor_tensor(out=parea[:], in0=pwx, in1=pwy,
                                    op=tt.mult)
            u1 = f32t(Tb, 'u1')
            nc.vector.tensor_tensor(out=u1[:], in0=parea[:], in1=tarea,
                                    op=tt.add)
            un = f32t(Tb, 'un')
            nc.vector.tensor_tensor(out=un[:], in0=u1[:], in1=inter[:],
                                    op=tt.subtract)
            ru = f32t(Tb, 'ru')
            nc.vector.reciprocal(ru[:], un[:])
            iou = f32t(Tb, 'iou')
            nc.vector.tensor_tensor(out=iou[:], in0=inter[:], in1=ru[:],
                                    op=tt.mult)
            ca0 = f32t(Tb, 'ca0')
            cwx, cwy = xy(cwh)
            nc.vector.tensor_tensor(out=ca0[:], in0=cwx, in1=cwy, op=tt.mult)
            ca = f32t(Tb, 'ca')
            nc.vector.tensor_scalar_add(ca[:], ca0[:], EPS)
            rc = f32t(Tb, 'rc')
            nc.vector.reciprocal(rc[:], ca[:])
            dif = f32t(Tb, 'dif')
            nc.vector.tensor_tensor(out=dif[:], in0=ca[:], in1=un[:],
                                    op=tt.subtract)
            dt = f32t(Tb, 'dt')
            nc.vector.tensor_tensor(out=dt[:], in0=dif[:], in1=rc[:],
                                    op=tt.mult)
            nc.vector.tensor_tensor(out=out_t[:, 0:Tb], in0=iou[:],
                                    in1=dt[:], op=tt.subtract)

            nc.scalar.dma_start(out_d.ap(), out_t[:])
    nc.compile()
    return nc


# --------------------------------------------------------------------------
# entry point
# --------------------------------------------------------------------------

def kernel(p0, p1, p2, targets):
    p0 = np.asarray(p0, np.float32)
    p1 = np.asarray(p1, np.float32)
    p2 = np.asarray(p2, np.float32)
    targets = np.asarray(targets, np.float32)
    p_list = [p0, p1, p2]
    bf16 = ml_dtypes.bfloat16

    levels = [_build_level(targets, l) for l in range(3)]

    # ---- active slot lists per level, round-robin over cores for balance
    lev = []
    for l in range(3):
        L = levels[l]
        H, W = LEVEL_HW[l]
        act = L['sl_ok'][:, None, :] & L['bmask'][None, :, :]   # [5, 3, M]
        ss, aa, mm = np.nonzero(act)
        n = len(ss)
        img = L['img'][mm]
        k = np.arange(n)
        core = k % N_CORES
        j = k // N_CORES
        celly = L['celly'][ss, mm]
        cellx = L['cellx'][ss, mm]
        p_r = p_list[l].reshape(N_IMG, A, 5 + NCLS, H, W)
        op85 = p_r[img, aa, :, celly, cellx]                    # [n, 85]
        lev.append(dict(n=n, aa=aa, img=img, core=core, j=j,
                        celly=celly, cellx=cellx, op85=op85, H=H, W=W,
                        ox=L['offx'][ss, mm], oy=L['offy'][ss, mm],
                        tw=L['tw'][mm], th=L['th'][mm],
                        anc=ANCHORS[l][aa],
                        cls_id=np.clip(L['cls_id'][mm], 0, NCLS - 1)))

    Ts = [max(1, int(-(-(-(-lev[l]['n'] // N_CORES)) // 128)))
          for l in range(3)]
    cumT = np.concatenate([[0], np.cumsum(Ts)])
    Tb = int(cumT[3])
    Tb2, B, W_SP, chunks, regions = _layout(Ts)
    assert Tb2 == Tb

    nc = _build_bass(Ts)
    OUTW = Tb + 6

    # ---- pack per-core device tensors
    spin = np.full((N_CORES, 128, W_SP), PAD_VAL, np.float32)
    boxd = np.zeros((N_CORES, 128, 7 * Tb), np.float32)
    boxd[:, :, 0:2 * Tb] = 1.0          # awh4 pad
    boxd[:, :, 4 * Tb:6 * Tb] = 1.0     # tc2 pad
    boxd[:, :, 6 * Tb:7 * Tb] = 1.0     # tarea pad

    # objectness planes (channel 4), contiguous per level
    base = B
    for l in range(3):
        H, W = LEVEL_HW[l]
        need = 128 * OBJ_COLS[l]
        for c in range(N_CORES):
            ob = np.ascontiguousarray(
                p_list[l][c * IMG_PER_CORE:(c + 1) * IMG_PER_CORE]
                .reshape(IMG_PER_CORE, A, 5 + NCLS, H, W)[:, :, 4]).reshape(-1)
            if len(ob) < need:
                ob = np.concatenate(
                    [ob, np.full(need - len(ob), PAD_VAL, np.float32)])
            spin[c, :, base:base + OBJ_COLS[l]] = ob.reshape(128, OBJ_COLS[l])
        base += OBJ_COLS[l]

    # matched-row logits + box data
    cls_s = B + OBJ_W
    for l in range(3):
        V = lev[l]
        if V['n'] == 0:
            continue
        p = V['j'] % 128
        t = V['j'] // 128
        u = cumT[l] + t
        core = V['core']
        e4 = np.arange(4)
        spin[core[:, None], p[:, None],
             u[:, None] * 4 + e4[None, :]] = -V['op85'][:, 0:4]
        cc = np.arange(NCLS)
        spin[core[:, None], p[:, None],
             cls_s + (cumT[l] + t)[:, None] * 80 + cc[None, :]] = \
            V['op85'][:, 5:]
        tw, th = V['tw'], V['th']
        ox, oy = V['ox'], V['oy']
        boxd[core, p, 2 * u] = 4.0 * V['anc'][:, 0]
        boxd[core, p, 2 * u + 1] = 4.0 * V['anc'][:, 1]
        boxd[core, p, 2 * Tb + 2 * u] = ox - tw * 0.5 + 0.5
        boxd[core, p, 2 * Tb + 2 * u + 1] = oy - th * 0.5 + 0.5
        boxd[core, p, 4 * Tb + 2 * u] = ox + tw * 0.5 + 0.5
        boxd[core, p, 4 * Tb + 2 * u + 1] = oy + th * 0.5 + 0.5
        boxd[core, p, 6 * Tb + u] = tw * th + EPS

    in_maps = [{'spin': spin[c].astype(bf16), 'box': boxd[c]}
               for c in range(N_CORES)]
    res = bass_utils.run_bass_kernel_spmd(nc, in_maps,
                                          core_ids=list(range(N_CORES)))
    global LAST_EXEC_NS, LAST_RESULT
    LAST_EXEC_NS = res.exec_time_ns
    LAST_RESULT = res
    outs = np.stack([res.results[c]['out'] for c in range(N_CORES)])

    # ---- host finalize
    total = 0.0
    for l in range(3):
        V = lev[l]
        H, W = LEVEL_HW[l]
        n = V['n']
        cnt = max(float(n), 1.0)
        obj_sum = outs[:, :, Tb + l].sum(dtype=np.float64)
        cls_sum = outs[:, :, Tb + 3 + l].sum(dtype=np.float64)
        corr = 0.0
        lbox_sum = 0.0
        xcls_sum = 0.0
        if n:
            p = V['j'] % 128
            u = cumT[l] + V['j'] // 128
            giou = outs[V['core'], p, u].astype(np.float64)
            lbox_sum = np.sum(1.0 - giou)
            xcls_sum = np.sum(V['op85'][np.arange(n), 5 + V['cls_id']]
                              .astype(np.float64))
            fk = ((V['img'].astype(np.int64) * A + V['aa']) * H
                  + V['celly']) * W + V['cellx']
            order = np.argsort(fk, kind='stable')
            fks = fk[order]
            vv = np.clip(giou[order], 0.0, None)
            xx = V['op85'][:, 4].astype(np.float64)[order]
            _, start = np.unique(fks, return_index=True)
            ymax = np.maximum.reduceat(vv, start)
            corr = np.sum(ymax * xx[start])
        count = N_IMG * A * H * W
        lb = lbox_sum / cnt
        lc = (cls_sum - xcls_sum) / (cnt * NCLS)
        lo = (obj_sum - corr) / count
        total += HYP_BOX * lb + HYP_CLS * lc + HYP_OBJ * BALANCE[l] * lo
    return np.float32(total * N_IMG)


LAST_EXEC_NS = None
LAST_RESULT = None
